# revision 19
# baseline (speedup 1.0000x reference)
import numpy as np

_CACHE = {}

N_CORES = 8
TOK = 16384
TOK_PER = TOK // N_CORES  # 2048 tokens per core
DIM = 2048
NE = 64
TOPK = 8
KC = 128            # contraction chunk (partition dim)
NK = DIM // KC      # 16 chunks
NT = 512            # token tile = one f32 PSUM bank
NJ = TOK_PER // NT  # 4 token tiles
WCOLS = NK * NE     # 1024 weight columns (all chunks, packed)
HK = NK // 2        # chunks per half-block DMA


def _build():
    import concourse.bass as bass
    from concourse import bacc, mybir

    nc = bacc.Bacc(
        "TRN2",
        target_bir_lowering=False,
        debug=False,
        enable_asserts=False,
        num_devices=N_CORES,
    )
    # j-major packed input:
    #   xpk[:, 0:WCOLS] = weights, wpk[p, k*NE + e] = W[e, k*KC + p]
    #   then for each token tile j, for each chunk k: [128, NT] block of
    #   x[t, :] with xblk[p, t'] = x[j*NT + t', k*KC + p]
    xpk = nc.dram_tensor(
        "xpk", (KC, WCOLS + NK * TOK_PER), mybir.dt.float16, kind="ExternalInput"
    ).ap()
    # packed output: rows 0:64 even-chunk half, 64:128 odd-chunk half
    out = nc.dram_tensor("opk", (KC, TOK_PER), mybir.dt.float16, kind="ExternalOutput").ap()

    # chunks per DMA, two DMAs per token tile. The final block is split
    # 12/4 so little matmul work remains after the last byte lands.
    splits = [(HK, HK)] * (NJ - 1) + [(12, 4)]

    # ---- static allocations ----
    halves = []  # 2 per j
    for j in range(NJ):
        for h in range(2):
            w = WCOLS if (j == 0 and h == 0) else 0
            t = nc.alloc_sbuf_tensor(
                f"xs{j}_{h}", [KC, w + splits[j][h] * NT], mybir.dt.float16
            )
            halves.append(t.ap())
    o128 = nc.alloc_sbuf_tensor("o128", [KC, TOK_PER], mybir.dt.float16).ap()
    accs = [nc.alloc_psum_tensor(f"acc{j}", [KC, NT], mybir.dt.float32).ap() for j in range(NJ)]

    s_in = [nc.alloc_semaphore(f"sin{i}") for i in range(2 * NJ)]
    s_acc = nc.alloc_semaphore("sacc")
    s_cast = nc.alloc_semaphore("scast")
    s_out = nc.alloc_semaphore("sout")
    all_sems = s_in + [s_acc, s_cast, s_out]
    lo = min(s.num for s in all_sems)
    hi = max(s.num for s in all_sems)

    # No startup sem clear needed: sems are zero at NEFF load, and the
    # runtime's end-of-execution postamble zeroes the whole kernel sem
    # range after every run.
    _ = (lo, hi)

    # ---- input DMAs (HWDGE / sync ring), issued up front ----
    col = 0
    for j in range(NJ):
        for h in range(2):
            i = 2 * j + h
            w = WCOLS if i == 0 else 0
            n = w + splits[j][h] * NT
            nc.sync.dma_start(halves[i][:], xpk[:, col:col + n]).then_inc(s_in[i], 16)
            col += n
    wsb = halves[0]

    # ---- matmuls: per token tile j, 16 chunks; even chunk -> PE col
    # group 0 (PSUM rows 0:64), odd -> group 1 (rows 64:128). Consecutive
    # same-group matmuls use different weights, so LDWEIGHTS pipelines. ----
    for j in range(NJ):
        for h in range(2):
            i = 2 * j + h
            nc.tensor.wait_ge(s_in[i], 16)
            xt = halves[i]
            xoff = WCOLS if i == 0 else 0
            k0 = splits[j][0] if h == 1 else 0
            for kk in range(splits[j][h]):
                k = k0 + kk
                g = k % 2
                mm = nc.tensor.matmul(
                    accs[j][g * NE:(g + 1) * NE, :],
                    wsb[:, k * NE:(k + 1) * NE],
                    xt[:, xoff + kk * NT:xoff + (kk + 1) * NT],
                    start=(k < 2),
                    stop=(k >= NK - 2),
                )
            if h == 1:
                mm.then_inc(s_acc, 1)
        # PSUM -> SBUF cast for this token tile (DVE), then store it
        nc.vector.wait_ge(s_acc, j + 1)
        nc.vector.tensor_copy(o128[:, j * NT:(j + 1) * NT], accs[j][:]).then_inc(s_cast, 1)
        eng = nc.scalar if j % 2 == 0 else nc.sync
        eng.wait_ge(s_cast, j + 1)
        eng.dma_start(out[:, j * NT:(j + 1) * NT], o128[:, j * NT:(j + 1) * NT]).then_inc(s_out, 16)
    nc.sync.wait_ge(s_out, 16 * NJ)
    nc.compile()
    return nc


def _make_in_maps(x, W):
    x = np.asarray(x, dtype=np.float32)
    W = np.asarray(W, dtype=np.float32)
    WT = W.T.astype(np.float16)  # [DIM, NE]
    wpk = WT.reshape(NK, KC, NE).transpose(1, 0, 2).reshape(KC, WCOLS)
    in_maps = []
    for i in range(N_CORES):
        xs = x[i * TOK_PER:(i + 1) * TOK_PER]
        xT = xs.T.astype(np.float16)  # [DIM, TOK_PER]
        # [NK, KC, NJ, NT] -> [KC, NJ, NK, NT]
        xp = (
            xT.reshape(NK, KC, NJ, NT)
            .transpose(1, 2, 0, 3)
            .reshape(KC, NK * TOK_PER)
        )
        in_maps.append({"xpk": np.ascontiguousarray(np.concatenate([wpk, xp], axis=1))})
    return in_maps


def kernel(x, W):
    from concourse import bass_utils

    if "nc" not in _CACHE:
        _CACHE["nc"] = _build()
    nc = _CACHE["nc"]

    in_maps = _make_in_maps(x, W)
    res = bass_utils.run_bass_kernel_spmd(nc, in_maps, list(range(N_CORES)))
    parts = []
    for r in res.results:
        o = np.asarray(r["opk"], dtype=np.float32)  # [128, TOK_PER]
        parts.append((o[:NE, :] + o[NE:, :]).T)     # [TOK_PER, NE]
    logits = np.concatenate(parts, axis=0)

    m = logits.max(axis=-1, keepdims=True)
    e = np.exp(logits - m)
    scores = e / e.sum(axis=-1, keepdims=True)
    idx = np.argsort(-scores, axis=-1, kind="stable")[:, :TOPK].astype(np.int32)
    w = np.take_along_axis(scores, idx, axis=-1).astype(np.float32)
    return w, idx


# revision 20
# speedup vs baseline: 1.1416x; 1.1416x over previous
import numpy as np

_CACHE = {}

N_CORES = 8
TOK = 16384
TOK_PER = TOK // N_CORES  # 2048 tokens per core
DIM = 2048
NE = 64
TOPK = 8
KC = 128            # contraction chunk (partition dim)
NK = DIM // KC      # 16 chunks
NT = 512            # token tile = one f32 PSUM bank
NJ = TOK_PER // NT  # 4 token tiles
WCOLS = NK * NE     # 1024 weight columns (all chunks, packed)
HK = NK // 2        # chunks per half-block DMA


def _build():
    import concourse.bass as bass
    from concourse import bacc, mybir

    nc = bacc.Bacc(
        "TRN2",
        target_bir_lowering=False,
        debug=False,
        enable_asserts=False,
        num_devices=N_CORES,
    )
    # j-major packed input:
    #   xpk[:, 0:WCOLS] = weights, wpk[p, k*NE + e] = W[e, k*KC + p]
    #   then for each token tile j, for each chunk k: [128, NT] block of
    #   x[t, :] with xblk[p, t'] = x[j*NT + t', k*KC + p]
    xpk = nc.dram_tensor(
        "xpk", (KC, WCOLS + NK * TOK_PER), mybir.dt.float16, kind="ExternalInput"
    ).ap()
    # packed output: rows 0:64 even-chunk half, 64:128 odd-chunk half
    out = nc.dram_tensor("opk", (KC, TOK_PER), mybir.dt.float16, kind="ExternalOutput").ap()

    # chunks per DMA, two DMAs per token tile. The final block is split
    # 12/4 so little matmul work remains after the last byte lands.
    splits = [(HK, HK)] * (NJ - 1) + [(12, 4)]

    # ---- static allocations ----
    halves = []  # 2 per j
    for j in range(NJ):
        for h in range(2):
            w = WCOLS if (j == 0 and h == 0) else 0
            t = nc.alloc_sbuf_tensor(
                f"xs{j}_{h}", [KC, w + splits[j][h] * NT], mybir.dt.float16
            )
            halves.append(t.ap())
    o128 = nc.alloc_sbuf_tensor("o128", [KC, TOK_PER], mybir.dt.float16).ap()
    accs = [nc.alloc_psum_tensor(f"acc{j}", [KC, NT], mybir.dt.float32).ap() for j in range(NJ)]
    # scratch bank for keep-warm matmuls (results never read)
    warm = nc.alloc_psum_tensor("warm", [NE, KC], mybir.dt.float32).ap()

    s_in = [nc.alloc_semaphore(f"sin{i}") for i in range(2 * NJ)]
    s_acc = nc.alloc_semaphore("sacc")
    s_cast = nc.alloc_semaphore("scast")
    s_out = nc.alloc_semaphore("sout")
    all_sems = s_in + [s_acc, s_cast, s_out]
    lo = min(s.num for s in all_sems)
    hi = max(s.num for s in all_sems)

    # No startup sem clear needed: sems are zero at NEFF load, and the
    # runtime's end-of-execution postamble zeroes the whole kernel sem
    # range after every run.
    _ = (lo, hi)

    # ---- input DMAs (HWDGE / sync ring), issued up front ----
    col = 0
    for j in range(NJ):
        for h in range(2):
            i = 2 * j + h
            w = WCOLS if i == 0 else 0
            n = w + splits[j][h] * NT
            nc.sync.dma_start(halves[i][:], xpk[:, col:col + n]).then_inc(s_in[i], 16)
            col += n
    wsb = halves[0]

    # ---- matmuls: per token tile j, 16 chunks; even chunk -> PE col
    # group 0 (PSUM rows 0:64), odd -> group 1 (rows 64:128). Consecutive
    # same-group matmuls use different weights, so LDWEIGHTS pipelines. ----
    for j in range(NJ):
        for h in range(2):
            i = 2 * j + h
            nc.tensor.wait_ge(s_in[i], 16)
            xt = halves[i]
            xoff = WCOLS if i == 0 else 0
            k0 = splits[j][0] if h == 1 else 0
            for kk in range(splits[j][h]):
                k = k0 + kk
                g = k % 2
                mm = nc.tensor.matmul(
                    accs[j][g * NE:(g + 1) * NE, :],
                    wsb[:, k * NE:(k + 1) * NE],
                    xt[:, xoff + kk * NT:xoff + (kk + 1) * NT],
                    start=(k < 2),
                    stop=(k >= NK - 2),
                )
            if h == 1:
                mm.then_inc(s_acc, 1)
        # PSUM -> SBUF cast for this token tile (DVE), then store it
        nc.vector.wait_ge(s_acc, j + 1)
        nc.vector.tensor_copy(o128[:, j * NT:(j + 1) * NT], accs[j][:]).then_inc(s_cast, 1)
        eng = nc.scalar if j % 2 == 0 else nc.sync
        eng.wait_ge(s_cast, j + 1)
        eng.dma_start(out[:, j * NT:(j + 1) * NT], o128[:, j * NT:(j + 1) * NT]).then_inc(s_out, 16)
    nc.sync.wait_ge(s_out, 16 * NJ)
    nc.compile()
    return nc


def _make_in_maps(x, W):
    x = np.asarray(x, dtype=np.float32)
    W = np.asarray(W, dtype=np.float32)
    WT = W.T.astype(np.float16)  # [DIM, NE]
    wpk = WT.reshape(NK, KC, NE).transpose(1, 0, 2).reshape(KC, WCOLS)
    in_maps = []
    for i in range(N_CORES):
        xs = x[i * TOK_PER:(i + 1) * TOK_PER]
        xT = xs.T.astype(np.float16)  # [DIM, TOK_PER]
        # [NK, KC, NJ, NT] -> [KC, NJ, NK, NT]
        xp = (
            xT.reshape(NK, KC, NJ, NT)
            .transpose(1, 2, 0, 3)
            .reshape(KC, NK * TOK_PER)
        )
        in_maps.append({"xpk": np.ascontiguousarray(np.concatenate([wpk, xp], axis=1))})
    return in_maps


def kernel(x, W):
    from concourse import bass_utils

    if "nc" not in _CACHE:
        _CACHE["nc"] = _build()
    nc = _CACHE["nc"]

    in_maps = _make_in_maps(x, W)
    res = bass_utils.run_bass_kernel_spmd(nc, in_maps, list(range(N_CORES)))
    parts = []
    for r in res.results:
        o = np.asarray(r["opk"], dtype=np.float32)  # [128, TOK_PER]
        parts.append((o[:NE, :] + o[NE:, :]).T)     # [TOK_PER, NE]
    logits = np.concatenate(parts, axis=0)

    m = logits.max(axis=-1, keepdims=True)
    e = np.exp(logits - m)
    scores = e / e.sum(axis=-1, keepdims=True)
    idx = np.argsort(-scores, axis=-1, kind="stable")[:, :TOPK].astype(np.int32)
    w = np.take_along_axis(scores, idx, axis=-1).astype(np.float32)
    return w, idx


# revision 21
# speedup vs baseline: 1.1677x; 1.0229x over previous
import numpy as np

_CACHE = {}

N_CORES = 8
TOK = 16384
TOK_PER = TOK // N_CORES  # 2048 tokens per core
DIM = 2048
NE = 64
TOPK = 8
KC = 128            # contraction chunk (partition dim)
NK = DIM // KC      # 16 chunks
NT = 512            # token tile = one f32 PSUM bank
NJ = TOK_PER // NT  # 4 token tiles
WCOLS = NK * NE     # 1024 weight columns (all chunks, packed)
HK = NK // 2        # chunks per half-block DMA


def _build():
    import concourse.bass as bass
    from concourse import bacc, mybir

    nc = bacc.Bacc(
        "TRN2",
        target_bir_lowering=False,
        debug=False,
        enable_asserts=False,
        num_devices=N_CORES,
    )
    # j-major packed input:
    #   xpk[:, 0:WCOLS] = weights, wpk[p, k*NE + e] = W[e, k*KC + p]
    #   then for each token tile j, for each chunk k: [128, NT] block of
    #   x[t, :] with xblk[p, t'] = x[j*NT + t', k*KC + p]
    xpk = nc.dram_tensor(
        "xpk", (KC, WCOLS + NK * TOK_PER), mybir.dt.float16, kind="ExternalInput"
    ).ap()
    # packed output: rows 0:64 even-chunk half, 64:128 odd-chunk half
    out = nc.dram_tensor("opk", (KC, TOK_PER), mybir.dt.float16, kind="ExternalOutput").ap()

    # chunks per DMA, two DMAs per token tile. The final block is split
    # 12/4 so little matmul work remains after the last byte lands.
    splits = [(HK, HK)] * (NJ - 1) + [(12, 4)]

    # ---- static allocations ----
    halves = []  # 2 per j
    for j in range(NJ):
        for h in range(2):
            w = WCOLS if (j == 0 and h == 0) else 0
            t = nc.alloc_sbuf_tensor(
                f"xs{j}_{h}", [KC, w + splits[j][h] * NT], mybir.dt.float16
            )
            halves.append(t.ap())
    o128 = nc.alloc_sbuf_tensor("o128", [KC, TOK_PER], mybir.dt.float16).ap()
    accs = [nc.alloc_psum_tensor(f"acc{j}", [KC, NT], mybir.dt.float32).ap() for j in range(NJ)]
    # scratch bank for keep-warm matmuls (results never read)
    warm = nc.alloc_psum_tensor("warm", [NE, KC], mybir.dt.float32).ap()

    s_in = [nc.alloc_semaphore(f"sin{i}") for i in range(2 * NJ)]
    s_acc = nc.alloc_semaphore("sacc")
    s_cast = nc.alloc_semaphore("scast")
    s_out = nc.alloc_semaphore("sout")
    all_sems = s_in + [s_acc, s_cast, s_out]
    lo = min(s.num for s in all_sems)
    hi = max(s.num for s in all_sems)

    # No startup sem clear needed: sems are zero at NEFF load, and the
    # runtime's end-of-execution postamble zeroes the whole kernel sem
    # range after every run.
    _ = (lo, hi)

    # ---- input DMAs (HWDGE / sync ring), issued up front ----
    col = 0
    for j in range(NJ):
        for h in range(2):
            i = 2 * j + h
            w = WCOLS if i == 0 else 0
            n = w + splits[j][h] * NT
            nc.sync.dma_start(halves[i][:], xpk[:, col:col + n]).then_inc(s_in[i], 16)
            col += n
    wsb = halves[0]

    # ---- matmuls: per token tile j, 16 chunks; even chunk -> PE col
    # group 0 (PSUM rows 0:64), odd -> group 1 (rows 64:128). Consecutive
    # same-group matmuls use different weights, so LDWEIGHTS pipelines. ----
    # Keep-warm matmuls: the PE's HAM clock gate drops it to 1.2 GHz after
    # idle windows; DMA-paced gaps between token-tile blocks keep it cold
    # (measured: K=4/8 for most of the kernel, 634 ns/MM instead of 379).
    # Filling the gaps with throwaway matmuls holds K=8/8. Results land in
    # a scratch bank and are never read; alternating weights let them
    # pipeline. Placed only where the PE is provably ahead of the stream.
    def _keep_warm(n):
        for dk in range(n):
            nc.tensor.matmul(
                warm[:, :],
                wsb[:, (dk % 2) * NE:((dk % 2) + 1) * NE],
                wsb[:, 0:KC],
                start=True,
                stop=True,
            )

    for j in range(NJ):
        for h in range(2):
            i = 2 * j + h
            nc.tensor.wait_ge(s_in[i], 16)
            xt = halves[i]
            xoff = WCOLS if i == 0 else 0
            k0 = splits[j][0] if h == 1 else 0
            for kk in range(splits[j][h]):
                k = k0 + kk
                g = k % 2
                mm = nc.tensor.matmul(
                    accs[j][g * NE:(g + 1) * NE, :],
                    wsb[:, k * NE:(k + 1) * NE],
                    xt[:, xoff + kk * NT:xoff + (kk + 1) * NT],
                    start=(k < 2),
                    stop=(k >= NK - 2),
                )
            if h == 1:
                mm.then_inc(s_acc, 1)
            if j == NJ - 1 and h == 0:
                _keep_warm(4)
        if j in (1, 2):
            _keep_warm(8)
        # PSUM -> SBUF cast for this token tile (DVE), then store it
        nc.vector.wait_ge(s_acc, j + 1)
        nc.vector.tensor_copy(o128[:, j * NT:(j + 1) * NT], accs[j][:]).then_inc(s_cast, 1)
        eng = nc.scalar if j % 2 == 0 else nc.sync
        eng.wait_ge(s_cast, j + 1)
        eng.dma_start(out[:, j * NT:(j + 1) * NT], o128[:, j * NT:(j + 1) * NT]).then_inc(s_out, 16)
    nc.sync.wait_ge(s_out, 16 * NJ)
    nc.compile()
    return nc


def _make_in_maps(x, W):
    x = np.asarray(x, dtype=np.float32)
    W = np.asarray(W, dtype=np.float32)
    WT = W.T.astype(np.float16)  # [DIM, NE]
    wpk = WT.reshape(NK, KC, NE).transpose(1, 0, 2).reshape(KC, WCOLS)
    in_maps = []
    for i in range(N_CORES):
        xs = x[i * TOK_PER:(i + 1) * TOK_PER]
        xT = xs.T.astype(np.float16)  # [DIM, TOK_PER]
        # [NK, KC, NJ, NT] -> [KC, NJ, NK, NT]
        xp = (
            xT.reshape(NK, KC, NJ, NT)
            .transpose(1, 2, 0, 3)
            .reshape(KC, NK * TOK_PER)
        )
        in_maps.append({"xpk": np.ascontiguousarray(np.concatenate([wpk, xp], axis=1))})
    return in_maps


def kernel(x, W):
    from concourse import bass_utils

    if "nc" not in _CACHE:
        _CACHE["nc"] = _build()
    nc = _CACHE["nc"]

    in_maps = _make_in_maps(x, W)
    res = bass_utils.run_bass_kernel_spmd(nc, in_maps, list(range(N_CORES)))
    parts = []
    for r in res.results:
        o = np.asarray(r["opk"], dtype=np.float32)  # [128, TOK_PER]
        parts.append((o[:NE, :] + o[NE:, :]).T)     # [TOK_PER, NE]
    logits = np.concatenate(parts, axis=0)

    m = logits.max(axis=-1, keepdims=True)
    e = np.exp(logits - m)
    scores = e / e.sum(axis=-1, keepdims=True)
    idx = np.argsort(-scores, axis=-1, kind="stable")[:, :TOPK].astype(np.int32)
    w = np.take_along_axis(scores, idx, axis=-1).astype(np.float32)
    return w, idx


# revision 22
# speedup vs baseline: 1.1851x; 1.0149x over previous
import numpy as np

_CACHE = {}

N_CORES = 8
TOK = 16384
TOK_PER = TOK // N_CORES  # 2048 tokens per core
DIM = 2048
NE = 64
TOPK = 8
KC = 128            # contraction chunk (partition dim)
NK = DIM // KC      # 16 chunks
NT = 512            # token tile = one f32 PSUM bank
NJ = TOK_PER // NT  # 4 token tiles
WCOLS = NK * NE     # 1024 weight columns (all chunks, packed)
HK = NK // 2        # chunks per half-block DMA


def _build():
    import concourse.bass as bass
    from concourse import bacc, mybir

    nc = bacc.Bacc(
        "TRN2",
        target_bir_lowering=False,
        debug=False,
        enable_asserts=False,
        num_devices=N_CORES,
    )
    # j-major packed input:
    #   xpk[:, 0:WCOLS] = weights, wpk[p, k*NE + e] = W[e, k*KC + p]
    #   then for each token tile j, for each chunk k: [128, NT] block of
    #   x[t, :] with xblk[p, t'] = x[j*NT + t', k*KC + p]
    xpk = nc.dram_tensor(
        "xpk", (KC, WCOLS + NK * TOK_PER), mybir.dt.float16, kind="ExternalInput"
    ).ap()
    # packed output: rows 0:64 even-chunk half, 64:128 odd-chunk half
    out = nc.dram_tensor("opk", (KC, TOK_PER), mybir.dt.float16, kind="ExternalOutput").ap()

    # chunks per DMA, two DMAs per token tile. The final block is split
    # 12/4 so little matmul work remains after the last byte lands.
    splits = [(HK, HK)] * (NJ - 1) + [(12, 4)]

    # ---- static allocations ----
    halves = []  # 2 per j
    for j in range(NJ):
        for h in range(2):
            w = WCOLS if (j == 0 and h == 0) else 0
            t = nc.alloc_sbuf_tensor(
                f"xs{j}_{h}", [KC, w + splits[j][h] * NT], mybir.dt.float16
            )
            halves.append(t.ap())
    o128 = nc.alloc_sbuf_tensor("o128", [KC, TOK_PER], mybir.dt.float16).ap()
    accs = [nc.alloc_psum_tensor(f"acc{j}", [KC, NT], mybir.dt.float32).ap() for j in range(NJ)]
    # scratch bank for keep-warm matmuls (results never read)
    warm = nc.alloc_psum_tensor("warm", [NE, KC], mybir.dt.float32).ap()

    s_in = [nc.alloc_semaphore(f"sin{i}") for i in range(2 * NJ)]
    s_acc = nc.alloc_semaphore("sacc")
    s_cast = nc.alloc_semaphore("scast")
    s_out = nc.alloc_semaphore("sout")
    all_sems = s_in + [s_acc, s_cast, s_out]
    lo = min(s.num for s in all_sems)
    hi = max(s.num for s in all_sems)

    # No startup sem clear needed: sems are zero at NEFF load, and the
    # runtime's end-of-execution postamble zeroes the whole kernel sem
    # range after every run.
    _ = (lo, hi)

    # ---- input DMAs (HWDGE / sync ring), issued up front ----
    col = 0
    for j in range(NJ):
        for h in range(2):
            i = 2 * j + h
            w = WCOLS if i == 0 else 0
            n = w + splits[j][h] * NT
            nc.sync.dma_start(halves[i][:], xpk[:, col:col + n]).then_inc(s_in[i], 16)
            col += n
    wsb = halves[0]

    # ---- matmuls: per token tile j, 16 chunks; even chunk -> PE col
    # group 0 (PSUM rows 0:64), odd -> group 1 (rows 64:128). Consecutive
    # same-group matmuls use different weights, so LDWEIGHTS pipelines. ----
    # Keep-warm matmuls: the PE's HAM clock gate drops it to 1.2 GHz after
    # idle windows; DMA-paced gaps between token-tile blocks keep it cold
    # (measured: K=4/8 for most of the kernel, 634 ns/MM instead of 379).
    # Filling the gaps with throwaway matmuls holds K=8/8. Results land in
    # a scratch bank and are never read; alternating weights let them
    # pipeline. Placed only where the PE is provably ahead of the stream.
    def _keep_warm(n):
        for dk in range(n):
            nc.tensor.matmul(
                warm[:, :],
                wsb[:, (dk % 2) * NE:((dk % 2) + 1) * NE],
                wsb[:, 0:KC],
                start=True,
                stop=True,
            )

    for j in range(NJ):
        for h in range(2):
            i = 2 * j + h
            nc.tensor.wait_ge(s_in[i], 16)
            xt = halves[i]
            xoff = WCOLS if i == 0 else 0
            k0 = splits[j][0] if h == 1 else 0
            for kk in range(splits[j][h]):
                k = k0 + kk
                g = k % 2
                mm = nc.tensor.matmul(
                    accs[j][g * NE:(g + 1) * NE, :],
                    wsb[:, k * NE:(k + 1) * NE],
                    xt[:, xoff + kk * NT:xoff + (kk + 1) * NT],
                    start=(k < 2),
                    stop=(k >= NK - 2),
                )
            if h == 1:
                mm.then_inc(s_acc, 1)
        if j in (1, 2):
            _keep_warm(8)
        # PSUM -> SBUF cast for this token tile (DVE), then store it
        nc.vector.wait_ge(s_acc, j + 1)
        nc.vector.tensor_copy(o128[:, j * NT:(j + 1) * NT], accs[j][:]).then_inc(s_cast, 1)
        eng = nc.scalar if j % 2 == 0 else nc.sync
        eng.wait_ge(s_cast, j + 1)
        eng.dma_start(out[:, j * NT:(j + 1) * NT], o128[:, j * NT:(j + 1) * NT]).then_inc(s_out, 16)
    nc.sync.wait_ge(s_out, 16 * NJ)
    nc.compile()
    return nc


def _make_in_maps(x, W):
    x = np.asarray(x, dtype=np.float32)
    W = np.asarray(W, dtype=np.float32)
    WT = W.T.astype(np.float16)  # [DIM, NE]
    wpk = WT.reshape(NK, KC, NE).transpose(1, 0, 2).reshape(KC, WCOLS)
    in_maps = []
    for i in range(N_CORES):
        xs = x[i * TOK_PER:(i + 1) * TOK_PER]
        xT = xs.T.astype(np.float16)  # [DIM, TOK_PER]
        # [NK, KC, NJ, NT] -> [KC, NJ, NK, NT]
        xp = (
            xT.reshape(NK, KC, NJ, NT)
            .transpose(1, 2, 0, 3)
            .reshape(KC, NK * TOK_PER)
        )
        in_maps.append({"xpk": np.ascontiguousarray(np.concatenate([wpk, xp], axis=1))})
    return in_maps


def kernel(x, W):
    from concourse import bass_utils

    if "nc" not in _CACHE:
        _CACHE["nc"] = _build()
    nc = _CACHE["nc"]

    in_maps = _make_in_maps(x, W)
    res = bass_utils.run_bass_kernel_spmd(nc, in_maps, list(range(N_CORES)))
    parts = []
    for r in res.results:
        o = np.asarray(r["opk"], dtype=np.float32)  # [128, TOK_PER]
        parts.append((o[:NE, :] + o[NE:, :]).T)     # [TOK_PER, NE]
    logits = np.concatenate(parts, axis=0)

    m = logits.max(axis=-1, keepdims=True)
    e = np.exp(logits - m)
    scores = e / e.sum(axis=-1, keepdims=True)
    idx = np.argsort(-scores, axis=-1, kind="stable")[:, :TOPK].astype(np.int32)
    w = np.take_along_axis(scores, idx, axis=-1).astype(np.float32)
    return w, idx
